# revision 2
# baseline (speedup 1.0000x reference)
"""Trainium2 Bass kernel for LLFullObjectCondensation loss (N=80000, K=512, C=2).

Strategy (8 NeuronCores, data-parallel over hits):
  - Each core gets a 10000-hit shard (padded to 79*128=10112), laid out [128, 79].
  - P1: per-hit quantities (q, payload, weights) as full-width [128,79] ops.
  - P2: local per-object max of beta via masked one-hot tiles + running
        elementwise max, then 4 PE transposes + reductions -> Bloc[512].
  - P3: selection pass: I = (bm == Bloc broadcast); PE segment-sums of
        (x0, x1, q) under I -> local condensation-point candidates.
  - AllGather(8x2KB candidates) + strict-first-core argmax reduce -> global
        x_alpha, q_alpha, beta_alpha.
  - P5: heavy N_local x K block: d2 via PE matmul (contract dim 4 trick),
        sqrt on ACT, hinge via min(s,1) linearization, repulsion row-sums via
        PE (q as stationary), attraction/corrections via tensor_tensor_reduce
        row-gather + PE segment-sums with lhsT = bm (beta+1 compensated rhs).
  - AllReduce(sum) of all per-object partials, then on-chip assembly of the
        scalar loss.
"""
import sys
import numpy as np

for _p in ("/opt/trn_rl_repo", "/root/.axon_site/_ro/trn_rl_repo"):
    if _p not in sys.path:
        sys.path.append(_p)

N = 80000
K = 512
NCORES = 8
S = N // NCORES          # 10000 hits per core
P = 128
T = 79                   # tiles per core, T*P = 10112 >= S
SP = T * P
KB = K // P              # 4 k-blocks
EPS = 1e-9
SQ_BIAS = 2e-5           # reference uses 1e-6; extra margin absorbs PSUM
                         # rounding of the expanded |x|^2-2x.a+|a|^2 form so
                         # sqrt never sees a negative input (error analysis:
                         # <1e-7 relative on the total loss)

_CACHE = {}


def _build(cc_mode='all'):
    import concourse.bass as bass
    import concourse.bacc as bacc
    import concourse.mybir as mybir
    import concourse.tile as tile
    from concourse import masks

    f32 = mybir.dt.float32
    i32 = mybir.dt.int32
    AF = mybir.ActivationFunctionType
    OP = mybir.AluOpType

    nc = bacc.Bacc("TRN2", target_bir_lowering=False, debug=False,
                   num_devices=NCORES)

    di = {}
    def din(name, shape):
        di[name] = nc.dram_tensor(name, shape, f32, kind="ExternalInput")
        return di[name]

    din("beta_r", [P, T])
    din("cc", [P, T, 2])
    din("pE", [P, T])
    din("ppos", [P, T, 2])
    din("ptime", [P, T])
    din("pid", [P, T, 6])
    din("tE", [P, T])
    din("tpos", [P, T, 2])
    din("ttime", [P, T])
    din("tidx", [P, T])
    din("valid", [P, T])
    out_d = nc.dram_tensor("out", [1, 1], f32, kind="ExternalOutput")

    with tile.TileContext(nc) as tc:
        with (
            tc.tile_pool(name="const", bufs=1) as cpool,
            tc.tile_pool(name="io", bufs=1) as io,
            tc.tile_pool(name="dram", bufs=1, space="DRAM") as dram,
            tc.tile_pool(name="psA", bufs=2, space="PSUM") as psA,
            tc.tile_pool(name="acc", bufs=1, space="PSUM") as accp,
        ):
            # PSUM budget: psA 2 banks (transposes, shared tag), accp holds
            # repP+segP (2 banks); selP/sc1P/sc2P live in scoped pools.
            # ---------- constants ----------
            ident = cpool.tile([P, P], f32)
            masks.make_identity(nc, ident[:])
            iotaI = cpool.tile([P, K], i32)
            nc.gpsimd.iota(iotaI[:], pattern=[[1, K]], base=0,
                           channel_multiplier=0)
            iotaF = cpool.tile([P, K], f32)
            nc.vector.tensor_copy(iotaF[:], iotaI[:])
            onescol = cpool.tile([P, 1], f32)
            nc.vector.memset(onescol[:], 1.0)
            onesrow = cpool.tile([1, P], f32)
            nc.vector.memset(onesrow[:], 1.0)

            _cb = {}
            def cbias(val):
                """[128,1] constant column for activation bias operands."""
                if val not in _cb:
                    ct = cpool.tile([P, 1], f32, name=f"cb{len(_cb)}")
                    nc.vector.memset(ct[:], val)
                    _cb[val] = ct
                return _cb[val][:]

            # ---------- load inputs ----------
            sb = {}
            for name, h in di.items():
                t_sb = io.tile(list(h.shape), f32, name=f"sb_{name}")
                nc.sync.dma_start(t_sb[:], h.ap())
                sb[name] = t_sb

            # ---------- P1: per-hit prep (all [128,T]-wide ops) ----------
            V = nc.vector
            SC = nc.scalar
            GP = nc.gpsimd

            def wtile(name, shape=None, dtype=None):
                return io.tile(shape or [P, T], dtype or f32, name=name)
            u8 = mybir.dt.uint8

            beta = wtile("beta")
            V.tensor_scalar(beta[:], sb["beta_r"][:], 1e-6, 1.0 - 1e-6,
                            OP.max, OP.min)
            betap1 = wtile("betap1")
            SC.activation(betap1[:], beta[:], AF.Identity, bias=cbias(1.0))
            rb1 = wtile("rb1")
            V.reciprocal(rb1[:], betap1[:])
            onem = wtile("onem")
            SC.activation(onem[:], beta[:], AF.Identity, bias=cbias(1.0), scale=-1.0)
            recm = wtile("recm")
            V.reciprocal(recm[:], onem[:])
            ratio = wtile("ratio")
            V.tensor_tensor(ratio[:], betap1[:], recm[:], OP.mult)

            is_obj = wtile("is_obj")
            V.tensor_scalar(is_obj[:], sb["tidx"][:], 0.0, None, OP.is_ge)
            is_noise = wtile("is_noise")
            V.tensor_scalar(is_noise[:], sb["tidx"][:], -1.0, None, OP.is_equal)

            # energy weights w = relu(min(wr,1)) ; wr=(tE-0.5)/9.5
            wr = wtile("wr")
            SC.activation(wr[:], sb["tE"][:], AF.Identity, bias=cbias(-0.5 / 9.5),
                          scale=1.0 / 9.5)
            ew = wtile("ew")
            V.tensor_scalar(ew[:], wr[:], 1.0, 0.0, OP.min, OP.max)
            pw = wtile("pw")
            V.tensor_tensor(pw[:], beta[:], ew[:], OP.mult)
            V.tensor_tensor(pw[:], pw[:], is_obj[:], OP.mult)

            # --- energy loss pieces (pre-transcendental) ---
            ediff_r = wtile("ediff_r")
            V.tensor_tensor(ediff_r[:], sb["tE"][:], sb["pE"][:], OP.subtract)
            ediff = wtile("ediff")
            SC.activation(ediff[:], ediff_r[:], AF.Abs)
            ed2 = wtile("ed2")
            V.tensor_tensor(ed2[:], ediff[:], ediff[:], OP.mult)
            ed001 = wtile("ed001")
            SC.activation(ed001[:], ediff[:], AF.Copy, scale=0.001)

            # --- position loss pieces ---
            dpos = wtile("dpos", [P, T, 2])
            V.tensor_tensor(dpos[:], sb["tpos"][:], sb["ppos"][:], OP.subtract)
            V.tensor_tensor(dpos[:], dpos[:], dpos[:], OP.mult)
            d2p = wtile("d2p")
            V.tensor_tensor(d2p[:], dpos[:, :, 0], dpos[:, :, 1], OP.add)

            # --- timing loss pieces ---
            dtm = wtile("dtm")
            V.tensor_tensor(dtm[:], sb["ttime"][:], sb["ptime"][:], OP.subtract)
            adt = wtile("adt")
            SC.activation(adt[:], dtm[:], AF.Abs)
            dt2 = wtile("dt2")
            V.tensor_tensor(dt2[:], dtm[:], dtm[:], OP.mult)
            lint = wtile("lint")
            SC.activation(lint[:], adt[:], AF.Identity, bias=cbias(-4.0), scale=4.0)
            ltt = wtile("ltt", dtype=u8)
            V.tensor_scalar(ltt[:], adt[:], 2.0, None, OP.is_lt)
            ht = wtile("ht")
            V.select(ht[:], ltt[:], dt2[:], lint[:])
            yt = wtile("yt")
            SC.activation(yt[:], ht[:], AF.Copy, scale=1.0 / 6.0)

            # --- classification loss ---
            pid2 = wtile("pid2", [P, T, 6])
            V.tensor_tensor(pid2[:], sb["pid"][:], sb["pid"][:], OP.mult)
            cred = wtile("cred")
            V.tensor_reduce(cred[:], pid2[:], mybir.AxisListType.X, OP.add)

            # --- transcendental block: Exp, then Sqrt, then Ln (grouped to
            # limit ACT table swaps; per-engine issue is program-order) ---
            ex = wtile("ex")
            SC.activation(ex[:], ed2[:], AF.Exp, scale=-0.1)
            xp = wtile("xp")
            SC.activation(xp[:], d2p[:], AF.Sqrt, bias=cbias(0.01), scale=0.01)

            lnr = wtile("lnr")
            SC.activation(lnr[:], ratio[:], AF.Ln)
            # q = (0.5*ln(ratio))^2 + 0.1, zeroed on padding
            halfln = wtile("halfln")
            SC.activation(halfln[:], lnr[:], AF.Copy, scale=0.5)
            q = wtile("q")
            V.tensor_tensor(q[:], halfln[:], halfln[:], OP.mult)
            V.scalar_tensor_tensor(q[:], q[:], 0.1, sb["valid"][:],
                                   OP.add, OP.mult)

            # energy softclip
            ye = wtile("ye")
            V.tensor_tensor(ye[:], ex[:], ed001[:], OP.add)
            lnye = wtile("lnye")
            SC.activation(lnye[:], ye[:], AF.Ln, bias=cbias(1.0))
            gte = wtile("gte", dtype=u8)
            V.tensor_scalar(gte[:], ye[:], 1.0, None, OP.is_gt)
            esc = wtile("esc")
            V.select(esc[:], gte[:], lnye[:], ye[:])

            # position huber + softclip
            xp2 = wtile("xp2")
            V.tensor_tensor(xp2[:], xp[:], xp[:], OP.mult)
            linp = wtile("linp")
            SC.activation(linp[:], xp[:], AF.Identity, bias=cbias(-100.0), scale=20.0)
            ltp = wtile("ltp", dtype=u8)
            V.tensor_scalar(ltp[:], xp[:], 10.0, None, OP.is_lt)
            hp = wtile("hp")
            V.select(hp[:], ltp[:], xp2[:], linp[:])
            yp = wtile("yp")
            SC.activation(yp[:], hp[:], AF.Copy, scale=1.0 / 3.0)
            lnyp = wtile("lnyp")
            SC.activation(lnyp[:], yp[:], AF.Ln, bias=cbias(1.0))
            gtp = wtile("gtp", dtype=u8)
            V.tensor_scalar(gtp[:], yp[:], 1.0, None, OP.is_gt)
            psc = wtile("psc")
            V.select(psc[:], gtp[:], lnyp[:], yp[:])

            # timing softclip
            lnyt = wtile("lnyt")
            SC.activation(lnyt[:], yt[:], AF.Ln, bias=cbias(1.0))
            gtt = wtile("gtt", dtype=u8)
            V.tensor_scalar(gtt[:], yt[:], 1.0, None, OP.is_gt)
            tsc = wtile("tsc")
            V.select(tsc[:], gtt[:], lnyt[:], yt[:])

            # payload = 10*esc + 3*psc + 6*tsc + (1e-8/6)*cred
            esc10 = wtile("esc10")
            SC.activation(esc10[:], esc[:], AF.Copy, scale=10.0)
            pay = wtile("pay")
            V.scalar_tensor_tensor(pay[:], psc[:], 3.0, esc10[:],
                                   OP.mult, OP.add)
            V.scalar_tensor_tensor(pay[:], tsc[:], 6.0, pay[:],
                                   OP.mult, OP.add)
            V.scalar_tensor_tensor(pay[:], cred[:], 1e-8 / 6.0, pay[:],
                                   OP.mult, OP.add)
            paypw = wtile("paypw")
            V.tensor_tensor(paypw[:], pay[:], pw[:], OP.mult)

            # selection rhs: [x0, x1, q]
            sel3 = wtile("sel3", [P, T, 3])
            SC.activation(sel3[:, :, 0:2], sb["cc"][:], AF.Copy)
            V.tensor_copy(sel3[:, :, 2], q[:])

            # d2-matmul lhsT quantities [-2x0, -2x1, 1, |x|^2] packed [P,T,4]
            prep4 = wtile("prep4", [P, T, 4])
            SC.activation(prep4[:, :, 0:2], sb["cc"][:], AF.Copy, scale=-2.0)
            V.memset(prep4[:, :, 2], 1.0)
            ccsq = wtile("ccsq", [P, T, 2])
            V.tensor_tensor(ccsq[:], sb["cc"][:], sb["cc"][:], OP.mult)
            V.tensor_tensor(prep4[:, :, 3], ccsq[:, :, 0], ccsq[:, :, 1],
                            OP.add)

            # extras: [noise*beta, noise, |x|^2, q] free-reduced to [P,4]
            extras = io.tile([P, 4], f32, name="extras")
            nb_t = wtile("nb_t")
            V.tensor_tensor(nb_t[:], is_noise[:], beta[:], OP.mult)
            V.tensor_reduce(extras[:, 0:1], nb_t[:], mybir.AxisListType.X, OP.add)
            V.tensor_reduce(extras[:, 1:2], is_noise[:], mybir.AxisListType.X, OP.add)
            V.tensor_reduce(extras[:, 2:3], prep4[:, :, 3], mybir.AxisListType.X, OP.add)
            V.tensor_reduce(extras[:, 3:4], q[:], mybir.AxisListType.X, OP.add)

            # ---------- P2: local per-object beta max ----------
            runmax = io.tile([P, K], f32, name="runmax")
            V.memset(runmax[:], 0.0)
            with tc.tile_pool(name="bmp", bufs=3) as bmp:
                for t in range(T):
                    bm = bmp.tile([P, K], f32, name="bm")
                    GP.tensor_scalar(bm[:], iotaF[:], sb["tidx"][:, t:t + 1],
                                     beta[:, t:t + 1], OP.is_equal, OP.mult)
                    V.tensor_tensor(runmax[:], runmax[:], bm[:], OP.max)

            # partition-reduce runmax -> Bloc [128,4] (k = 128*b + p)
            Bloc = io.tile([P, KB], f32, name="Bloc")
            for b in range(KB):
                tp = psA.tile([P, P], f32, name="tpose", tag="tpose")
                nc.tensor.transpose(tp[:], runmax[:, b * P:(b + 1) * P], ident[:])
                V.reduce_max(Bloc[:, b:b + 1], tp[:], axis=mybir.AxisListType.X)

            # free-layout broadcast copy of Bloc: BlocB[p, k] = Bloc[k%128, k//128]
            BlocF = io.tile([1, K], f32, name="BlocF")
            for b in range(KB):
                nc.sync.dma_start(BlocF[0:1, b * P:(b + 1) * P], Bloc[:, b:b + 1])
            # broadcast [1,K] across partitions via PE: ones[1,P].T @ BlocF
            BlocB = io.tile([P, K], f32, name="BlocB")
            with tc.tile_pool(name="bcp", bufs=1, space="PSUM") as bcp:
                blocps = bcp.tile([P, K], f32, name="blocps")
                nc.tensor.matmul(blocps[:], onesrow[:], BlocF[:],
                                 start=True, stop=True)
                SC.activation(BlocB[:], blocps[:], AF.Copy)

            # ---------- P3: selection segment-sums ----------
            with (
                tc.tile_pool(name="selpp", bufs=1, space="PSUM") as selpp,
                tc.tile_pool(name="bmp3", bufs=3) as bmp3,
            ):
                selP = selpp.tile([P, KB, 3], f32, name="selP")
                V.memset(selP[:], 0.0)
                for t in range(T):
                    bm = bmp3.tile([P, K], f32, name="bm3")
                    GP.tensor_scalar(bm[:], iotaF[:], sb["tidx"][:, t:t + 1],
                                     beta[:, t:t + 1], OP.is_equal, OP.mult)
                    Isel = bmp3.tile([P, K], f32, name="Isel")
                    V.tensor_tensor(Isel[:], bm[:], BlocB[:], OP.is_equal)
                    for b in range(KB):
                        nc.tensor.matmul(selP[:, b, :],
                                         Isel[:, b * P:(b + 1) * P],
                                         sel3[:, t, :],
                                         start=False, stop=(t == T - 1),
                                         skip_group_check=True)

                selsb = io.tile([P, KB, 3], f32, name="selsb")
                SC.activation(selsb[:], selP[:], AF.Copy)

            # ---------- P4: AllReduce-max of Bloc, then AllReduce-add of
            # equality-gated selection sums (winner core contributes) ----------
            arm_in = dram.tile([1, K], f32, name="arm_in")
            arm_out = dram.tile([1, K], f32, name="arm_out", addr_space="Shared")
            nc.sync.dma_start(arm_in[0:1, :], Bloc[:, :])     # p-outer pack
            if cc_mode in ('all', 'first', 'two'):
                nc.gpsimd.collective_compute(
                    "AllReduce", OP.max,
                    replica_groups=[list(range(NCORES))],
                    ins=[arm_in[:]], outs=[arm_out[:]],
                )
            else:
                nc.sync.dma_start(arm_out[:], arm_in[:])
            BglobB = io.tile([P, KB], f32, name="BglobB")
            nc.sync.dma_start(
                BglobB[:],
                arm_out[0:1, :].rearrange("o (p b) -> (o p) b", p=P))
            keep = io.tile([P, KB], f32, name="keep")
            V.tensor_tensor(keep[:], Bloc[:], BglobB[:], OP.is_equal)
            sel_c = io.tile([P, KB, 3], f32, name="sel_c")
            for qq in range(3):
                V.tensor_tensor(sel_c[:, :, qq], selsb[:, :, qq], keep[:],
                                OP.mult)
            ar2_in = dram.tile([1, 3 * K], f32, name="ar2_in")
            ar2_out = dram.tile([1, 3 * K], f32, name="ar2_out",
                                addr_space="Shared")
            nc.sync.dma_start(ar2_in[0:1, :], sel_c[:])       # p-outer pack
            if cc_mode in ('all', 'two'):
                nc.gpsimd.collective_compute(
                    "AllReduce", OP.add,
                    replica_groups=[list(range(NCORES))],
                    ins=[ar2_in[:]], outs=[ar2_out[:]],
                )
            else:
                nc.sync.dma_start(ar2_out[:], ar2_in[:])
            sel_g = io.tile([P, KB, 3], f32, name="sel_g")
            nc.sync.dma_start(
                sel_g[:],
                ar2_out[0:1, :].rearrange("o (p r) -> (o p) r", p=P))

            BstarB = BglobB[:]          # [P,KB]  beta_alpha (max beta)
            xa0B = sel_g[:, :, 0]
            xa1B = sel_g[:, :, 1]
            qaB = sel_g[:, :, 2]
            xasqB = io.tile([P, KB], f32, name="xasqB")
            tmpa = io.tile([P, KB], f32, name="tmpa")
            V.tensor_tensor(tmpa[:], xa0B, xa0B, OP.mult)
            V.tensor_tensor(xasqB[:], xa1B, xa1B, OP.mult)
            V.tensor_tensor(xasqB[:], xasqB[:], tmpa[:], OP.add)

            # free-layout rhs for the d2 matmul: rows [xa0; xa1; |xa|^2; 1]
            rhsD2 = io.tile([4, K], f32, name="rhsD2")
            V.memset(rhsD2[:], 1.0)       # row 3 stays 1.0; rows 0-2 DMA'd over
            for b in range(KB):
                nc.sync.dma_start(rhsD2[0:1, b * P:(b + 1) * P],
                                  sel_g[:, b:b + 1, 0])
                nc.sync.dma_start(rhsD2[1:2, b * P:(b + 1) * P],
                                  sel_g[:, b:b + 1, 1])
                nc.sync.dma_start(rhsD2[2:3, b * P:(b + 1) * P],
                                  xasqB[:, b:b + 1])

            # transpose prep4 -> lhsT4 [4, T, 128]: quantity r on partition r,
            # tile t's stationary operand = lhsT4[0:4, t, :] (base partition 0)
            lhsT4 = io.tile([4, T, P], f32, name="lhsT4")
            for r in range(4):
                tp = psA.tile([P, P], f32, name="tpose4", tag="tpose")
                nc.tensor.transpose(tp[0:T, :], prep4[:, :, r], ident[:])
                stage = io.tile([T, P], f32, name=f"tstage{r}")
                SC.activation(stage[:], tp[0:T, :], AF.Copy)
                nc.sync.dma_start(lhsT4[r:r + 1, :, :], stage[:])

            # ---------- P5 loop 1: d2 block, rep row-sums, self-distance ----------
            gstD = io.tile([P, T], f32, name="gstD")       # (beta+1)*d2_self
            repP = accp.tile([1, K], f32, name="repP")
            V.memset(repP[:], 0.0)
            scr = io.tile([P, K], f32, name="scr")         # ttr full-out scratch
            KH = K // 2
            with (
                tc.tile_pool(name="d2pool", bufs=2, space="PSUM") as d2pool,
                tc.tile_pool(name="sp", bufs=3) as sp,
                tc.tile_pool(name="bmp5", bufs=3) as bmp5,
            ):
                for t in range(T):
                    lhs_t = lhsT4[0:4, t, :]
                    d2ps = d2pool.tile([P, K], f32, name="d2ps")
                    nc.tensor.matmul(d2ps[:], lhs_t, rhsD2[:],
                                     start=True, stop=True)
                    bm = bmp5.tile([P, K], f32, name="bm5")
                    GP.tensor_scalar(bm[:], iotaF[:], sb["tidx"][:, t:t + 1],
                                     betap1[:, t:t + 1], OP.is_equal, OP.mult)
                    sS = sp.tile([P, K], f32, name="sS")
                    SC.activation(sS[:], d2ps[:], AF.Sqrt, bias=cbias(SQ_BIAS))
                    smv = sp.tile([P, K], f32, name="smv")
                    V.tensor_scalar(smv[:, 0:KH], sS[:, 0:KH], 1.0, None, OP.min)
                    GP.tensor_scalar(smv[:, KH:K], sS[:, KH:K], 1.0, None, OP.min)
                    nc.tensor.matmul(repP[:], q[:, t:t + 1], smv[:],
                                     start=False, stop=(t == T - 1),
                                     skip_group_check=True)
                    V.scalar_tensor_tensor(
                        scr[:], bm[:], 1.0, d2ps[:], OP.bypass, OP.mult,
                        accum_out=gstD[:, t:t + 1])

            # ---------- global per-hit math for segment rhs ----------
            qrb = wtile("qrb")
            V.tensor_tensor(qrb[:], q[:], rb1[:], OP.mult)
            G2 = wtile("G2")                    # d2_self
            V.tensor_tensor(G2[:], gstD[:], rb1[:], OP.mult)
            rhs_seg = io.tile([P, T, 6], f32, name="rhs_seg")
            # att' = q*d2_self/(b+1)
            V.tensor_tensor(rhs_seg[:, :, 0], G2[:], qrb[:], OP.mult)
            s2 = wtile("s2")
            SC.activation(s2[:], G2[:], AF.Sqrt, bias=cbias(SQ_BIAS))
            V.tensor_scalar(s2[:], s2[:], 1.0, None, OP.min)
            # qmin' = q*min(s,1)/(b+1)
            V.tensor_tensor(rhs_seg[:, :, 1], s2[:], qrb[:], OP.mult)
            V.tensor_tensor(rhs_seg[:, :, 2], sb["valid"][:], rb1[:], OP.mult)
            V.tensor_tensor(rhs_seg[:, :, 3], pw[:], rb1[:], OP.mult)
            V.tensor_tensor(rhs_seg[:, :, 4], paypw[:], rb1[:], OP.mult)
            V.tensor_copy(rhs_seg[:, :, 5], qrb[:])

            # ---------- P5 loop 2: segment sums with lhsT = bm ----------
            segP = accp.tile([P, KB, 6], f32, name="segP")
            V.memset(segP[:], 0.0)
            with tc.tile_pool(name="bmp6", bufs=3) as bmp6:
                for t in range(T):
                    bm = bmp6.tile([P, K], f32, name="bm6")
                    GP.tensor_scalar(bm[:], iotaF[:], sb["tidx"][:, t:t + 1],
                                     betap1[:, t:t + 1], OP.is_equal, OP.mult)
                    for b in range(KB):
                        nc.tensor.matmul(segP[:, b, :],
                                         bm[:, b * P:(b + 1) * P],
                                         rhs_seg[:, t, :],
                                         start=False, stop=(t == T - 1),
                                         skip_group_check=True)

            # ---------- P6: AllReduce of partials ----------
            segsb = io.tile([P, KB, 6], f32, name="segsb")
            SC.activation(segsb[:], segP[:], AF.Copy)
            repsb = io.tile([1, K], f32, name="repsb")
            SC.activation(repsb[:], repP[:], AF.Copy)

            NSEG = P * KB * 6
            NTOT = NSEG + K + 4 * P
            ar_in = dram.tile([1, NTOT], f32, name="ar_in")
            ar_out = dram.tile([1, NTOT], f32, name="ar_out", addr_space="Shared")
            nc.sync.dma_start(ar_in[0:1, 0:NSEG], segsb[:])
            nc.sync.dma_start(ar_in[0:1, NSEG:NSEG + K], repsb[:])
            nc.sync.dma_start(ar_in[0:1, NSEG + K:NTOT], extras[:])
            if cc_mode == 'all':
                nc.gpsimd.collective_compute(
                    "AllReduce", OP.add,
                    replica_groups=[list(range(NCORES))],
                    ins=[ar_in[:]], outs=[ar_out[:]],
                )
            else:
                nc.sync.dma_start(ar_out[:], ar_in[:])
            seg_g = io.tile([P, KB, 6], f32, name="seg_g")
            nc.sync.dma_start(
                seg_g[:],
                ar_out[0:1, 0:NSEG].rearrange("o (p r) -> (o p) r", p=P))
            extras_g = io.tile([P, 4], f32, name="extras_g")
            nc.sync.dma_start(
                extras_g[:],
                ar_out[0:1, NSEG + K:NTOT].rearrange("o (p r) -> (o p) r", p=P))
            repB = io.tile([P, KB], f32, name="repB")
            for b in range(KB):
                nc.sync.dma_start(
                    repB[:, b:b + 1],
                    ar_out[0:1, NSEG + b * P:NSEG + (b + 1) * P])

            # ---------- P7: assembly ----------
            # scalars: [nb, nn, xsq, qsum]
            scpp = tc.tile_pool(name="scpp", bufs=1, space="PSUM")
            scp = scpp.__enter__()
            sc1P = scp.tile([1, 4], f32, name="sc1P")
            nc.tensor.matmul(sc1P[:], onescol[:], extras_g[:],
                             start=True, stop=True)
            sc1 = io.tile([1, 4], f32, name="sc1")
            SC.activation(sc1[:], sc1P[:], AF.Copy)
            qsum_col = io.tile([P, 1], f32, name="qsum_col")
            qsps = scp.tile([P, 1], f32, name="qsps")
            nc.tensor.matmul(qsps[:], onesrow[:], sc1[0:1, 3:4],
                             start=True, stop=True)
            SC.activation(qsum_col[:], qsps[:], AF.Copy)

            attseg = seg_g[:, :, 0]
            qminseg = seg_g[:, :, 1]
            count = seg_g[:, :, 2]
            pwseg = seg_g[:, :, 3]
            payseg = seg_g[:, :, 4]
            qseg = seg_g[:, :, 5]

            def ntile(name):
                return io.tile([P, KB], f32, name=name)

            has = ntile("has")
            V.tensor_scalar(has[:], count[:], 0.0, None, OP.is_gt)
            rc = ntile("rc")        # 1/(count+eps)
            V.tensor_scalar(rc[:], count[:], EPS, None, OP.add)
            V.reciprocal(rc[:], rc[:])
            rnc = ntile("rnc")      # 1/(N-count+eps)
            V.tensor_scalar(rnc[:], count[:], -1.0, float(N) + EPS,
                            OP.mult, OP.add)
            V.reciprocal(rnc[:], rnc[:])

            la = ntile("la")        # qa*attseg/(count+eps) * has
            V.tensor_tensor(la[:], attseg[:], qaB, OP.mult)
            V.tensor_tensor(la[:], la[:], rc[:], OP.mult)
            V.tensor_tensor(la[:], la[:], has[:], OP.mult)

            # repfull_k = qsum_tot - repmm_k ; corr_k = qseg - qminseg
            # (repfull - corr) = qsum - repB - qseg + qminseg
            lr = ntile("lr")
            V.tensor_scalar(lr[:], repB[:], qsum_col[:], None, OP.subtract)
            V.tensor_tensor(lr[:], lr[:], qseg[:], OP.add)
            V.tensor_tensor(lr[:], lr[:], qminseg[:], OP.subtract)
            # now lr = repB - qsum + qseg - qminseg = -(repfull - corr)
            V.tensor_tensor(lr[:], lr[:], qaB, OP.mult)
            V.tensor_tensor(lr[:], lr[:], rnc[:], OP.mult)
            V.tensor_tensor(lr[:], lr[:], has[:], OP.mult)
            SC.activation(lr[:], lr[:], AF.Copy, scale=-1.0)

            lb = ntile("lb")        # has*(1 - beta_alpha)
            V.tensor_scalar(lb[:], BstarB, -1.0, 1.0, OP.mult, OP.add)
            V.tensor_tensor(lb[:], lb[:], has[:], OP.mult)

            lp = ntile("lp")        # has*paynum/(payden+eps)
            V.tensor_scalar(lp[:], pwseg[:], EPS, None, OP.add)
            V.reciprocal(lp[:], lp[:])
            V.tensor_tensor(lp[:], lp[:], payseg[:], OP.mult)
            V.tensor_tensor(lp[:], lp[:], has[:], OP.mult)

            asm = io.tile([P, 5], f32, name="asm")
            V.tensor_reduce(asm[:, 0:1], la[:], mybir.AxisListType.X, OP.add)
            V.tensor_reduce(asm[:, 1:2], lr[:], mybir.AxisListType.X, OP.add)
            V.tensor_reduce(asm[:, 2:3], lb[:], mybir.AxisListType.X, OP.add)
            V.tensor_reduce(asm[:, 3:4], lp[:], mybir.AxisListType.X, OP.add)
            V.tensor_reduce(asm[:, 4:5], has[:], mybir.AxisListType.X, OP.add)
            sc2P = scp.tile([1, 5], f32, name="sc2P")
            nc.tensor.matmul(sc2P[:], onescol[:], asm[:], start=True, stop=True)
            fin = io.tile([1, 5], f32, name="fin")
            SC.activation(fin[:], sc2P[:], AF.Copy)

            # total = (la+lr+lb+lp)/n_obj + nb/(nn+eps) + 0.001*xsq/(2N)
            s4 = io.tile([1, 1], f32, name="s4")
            V.tensor_reduce(s4[:], fin[0:1, 0:4], mybir.AxisListType.X, OP.add)
            nobj = io.tile([1, 1], f32, name="nobj")
            V.tensor_scalar(nobj[:], fin[0:1, 4:5], EPS, None, OP.add)
            V.reciprocal(nobj[:], nobj[:])
            tot = io.tile([1, 1], f32, name="tot")
            V.tensor_tensor(tot[:], s4[:], nobj[:], OP.mult)
            nden = io.tile([1, 1], f32, name="nden")
            V.tensor_scalar(nden[:], sc1[0:1, 1:2], EPS, None, OP.add)
            V.reciprocal(nden[:], nden[:])
            V.tensor_tensor(nden[:], nden[:], sc1[0:1, 0:1], OP.mult)
            V.tensor_tensor(tot[:], tot[:], nden[:], OP.add)
            lcc = io.tile([1, 1], f32, name="lcc")
            SC.activation(lcc[:], sc1[0:1, 2:3], AF.Copy,
                          scale=0.001 / (2.0 * N))
            V.tensor_tensor(tot[:], tot[:], lcc[:], OP.add)
            nc.sync.dma_start(out_d.ap(), tot[:])
            scpp.__exit__(None, None, None)

    nc.compile()
    return nc


def _host_prep(inputs):
    """Slice, pad and re-layout the full inputs into 8 per-core input maps."""
    def lay(a2):                       # [SP, w] -> [128, T, w]
        w = a2.shape[1]
        r = a2.reshape(T, P, w).transpose(1, 0, 2)
        return np.ascontiguousarray(r.astype(np.float32))

    in_maps = []
    for c in range(NCORES):
        sl = slice(c * S, (c + 1) * S)

        def pad(a, fill=0.0):
            out = np.full((SP, a.shape[1]), fill, np.float32)
            out[:S] = a[sl]
            return out

        tidx = np.full((SP, 1), -2.0, np.float32)
        tidx[:S, 0] = inputs["t_idx"][sl, 0].astype(np.float32)
        valid = np.zeros((SP, 1), np.float32)
        valid[:S] = 1.0
        m = {
            "beta_r": lay(pad(inputs["pred_beta"]))[:, :, 0],
            "cc": lay(pad(inputs["pred_ccoords"])),
            "pE": lay(pad(inputs["pred_energy"]))[:, :, 0],
            "ppos": lay(pad(inputs["pred_pos"])),
            "ptime": lay(pad(inputs["pred_time"]))[:, :, 0],
            "pid": lay(pad(inputs["pred_id"])),
            "tE": lay(pad(inputs["t_energy"]))[:, :, 0],
            "tpos": lay(pad(inputs["t_pos"])),
            "ttime": lay(pad(inputs["t_time"]))[:, :, 0],
            "tidx": lay(tidx)[:, :, 0],
            "valid": lay(valid)[:, :, 0],
        }
        m = {k: np.ascontiguousarray(v) for k, v in m.items()}
        in_maps.append(m)
    return in_maps


def _run(inputs, trace=False, tmpdir=None):
    from concourse import bass_utils
    if "nc" not in _CACHE:
        _CACHE["nc"] = _build()
    nc = _CACHE["nc"]
    in_maps = _host_prep(inputs)
    res = bass_utils.run_bass_kernel_spmd(
        nc, in_maps, core_ids=list(range(NCORES)), trace=trace, tmpdir=tmpdir)
    return res


def kernel(**inputs):
    res = _run(inputs, trace=False)
    val = np.float32(res.results[0]["out"][0, 0])
    return np.array(val, dtype=np.float32)[()]


if __name__ == "__main__":
    d = np.load("/tmp/inputs.npz")
    inp = {k: d[k] for k in d.files}
    print("kernel:", kernel(**inp))



# revision 13
# speedup vs baseline: 5.3920x; 5.3920x over previous
"""Trainium2 Bass kernel for LLFullObjectCondensation loss (N=80000, K=512, C=2).

Strategy (8 NeuronCores, data-parallel over hits):
  - Each core gets a 10000-hit shard (padded to 79*128=10112), laid out [128, 79].
  - P1: per-hit quantities (q, payload, weights) as full-width [128,79] ops.
  - P2: local per-object max of beta via masked one-hot tiles + running
        elementwise max, then 4 PE transposes + reductions -> Bloc[512].
  - P3: selection pass: I = (bm == Bloc broadcast); PE segment-sums of
        (x0, x1, q) under I -> local condensation-point candidates.
  - AllGather(8x2KB candidates) + strict-first-core argmax reduce -> global
        x_alpha, q_alpha, beta_alpha.
  - P5: heavy N_local x K block: d2 via PE matmul (contract dim 4 trick),
        sqrt on ACT, hinge via min(s,1) linearization, repulsion row-sums via
        PE (q as stationary), attraction/corrections via tensor_tensor_reduce
        row-gather + PE segment-sums with lhsT = bm (beta+1 compensated rhs).
  - AllReduce(sum) of all per-object partials, then on-chip assembly of the
        scalar loss.
"""
import sys
import numpy as np

for _p in ("/opt/trn_rl_repo", "/root/.axon_site/_ro/trn_rl_repo"):
    if _p not in sys.path:
        sys.path.append(_p)

N = 80000
K = 512
NCORES = 8
S = N // NCORES          # 10000 hits per core
P = 128
T = 79                   # tiles per core, T*P = 10112 >= S
SP = T * P
KB = K // P              # 4 k-blocks
EPS = 1e-9
SQ_BIAS = 1.5e-3         # reference uses 1e-6; extra margin absorbs fp32r
                         # matmul rounding of the expanded |x|^2-2x.a+|a|^2
                         # form so sqrt never sees a negative input (error
                         # analysis: ~1e-4 relative on the total loss)

_CACHE = {}


def _build(cc_mode='all'):
    import concourse.bass as bass
    import concourse.bacc as bacc
    import concourse.mybir as mybir
    import concourse.tile as tile
    from concourse import masks

    f32 = mybir.dt.float32
    f32r = mybir.dt.float32r
    f16 = mybir.dt.float16
    i32 = mybir.dt.int32
    AF = mybir.ActivationFunctionType
    OP = mybir.AluOpType

    nc = bacc.Bacc("TRN2", target_bir_lowering=False, debug=False,
                   num_devices=NCORES)

    di = {}
    def din(name, shape):
        di[name] = nc.dram_tensor(name, shape, f32, kind="ExternalInput")
        return di[name]

    din("beta_r", [P, T])
    din("cc", [P, T, 2])
    din("pE", [P, T])
    din("ppos", [P, T, 2])
    din("ptime", [P, T])
    din("pid", [P, T, 6])
    din("tE", [P, T])
    din("tpos", [P, T, 2])
    din("ttime", [P, T])
    din("tidx", [P, T])
    din("valid", [P, T])
    out_d = nc.dram_tensor("out", [1, 1], f32, kind="ExternalOutput")

    with tile.TileContext(nc) as tc:
        with (
            tc.tile_pool(name="const", bufs=1) as cpool,
            tc.tile_pool(name="io", bufs=1) as io,
            tc.tile_pool(name="dram", bufs=1, space="DRAM") as dram,
            tc.tile_pool(name="psA", bufs=2, space="PSUM") as psA,
            tc.tile_pool(name="acc", bufs=1, space="PSUM") as accp,
        ):
            # PSUM budget: psA 2 banks (transposes, shared tag), accp holds
            # repP+segP (2 banks); selP/sc1P/sc2P live in scoped pools.
            # ---------- constants ----------
            ident = cpool.tile([P, P], f32)
            masks.make_identity(nc, ident[:])
            iotaI = cpool.tile([P, K], i32)
            nc.gpsimd.iota(iotaI[:], pattern=[[1, K]], base=0,
                           channel_multiplier=0)
            iotaF = cpool.tile([P, K], f32)
            nc.vector.tensor_copy(iotaF[:], iotaI[:])
            onescol = cpool.tile([P, 1], f32)
            nc.vector.memset(onescol[:], 1.0)
            onesrow = cpool.tile([1, P], f32)
            nc.vector.memset(onesrow[:], 1.0)

            _cb = {}
            def cbias(val):
                """[128,1] constant column for activation bias operands."""
                if val not in _cb:
                    ct = cpool.tile([P, 1], f32, name=f"cb{len(_cb)}")
                    nc.vector.memset(ct[:], val)
                    _cb[val] = ct
                return _cb[val][:]

            # ---------- load inputs ----------
            sb = {}
            for name, h in di.items():
                t_sb = io.tile(list(h.shape), f32, name=f"sb_{name}")
                nc.sync.dma_start(t_sb[:], h.ap())
                sb[name] = t_sb

            # ---------- P1: per-hit prep (all [128,T]-wide ops) ----------
            V = nc.vector
            SC = nc.scalar
            GP = nc.gpsimd

            def wtile(name, shape=None, dtype=None):
                return io.tile(shape or [P, T], dtype or f32, name=name)
            u8 = mybir.dt.uint8

            beta = wtile("beta")
            V.tensor_scalar(beta[:], sb["beta_r"][:], 1e-6, 1.0 - 1e-6,
                            OP.max, OP.min)
            betap1 = wtile("betap1")
            SC.activation(betap1[:], beta[:], AF.Identity, bias=cbias(1.0))
            rb1 = wtile("rb1")
            V.reciprocal(rb1[:], betap1[:])
            onem = wtile("onem")
            SC.activation(onem[:], beta[:], AF.Identity, bias=cbias(1.0), scale=-1.0)
            recm = wtile("recm")
            V.reciprocal(recm[:], onem[:])
            ratio = wtile("ratio")
            V.tensor_tensor(ratio[:], betap1[:], recm[:], OP.mult)

            is_obj = wtile("is_obj")
            V.tensor_scalar(is_obj[:], sb["tidx"][:], 0.0, None, OP.is_ge)
            is_noise = wtile("is_noise")
            V.tensor_scalar(is_noise[:], sb["tidx"][:], -1.0, None, OP.is_equal)

            # energy weights w = relu(min(wr,1)) ; wr=(tE-0.5)/9.5
            wr = wtile("wr")
            SC.activation(wr[:], sb["tE"][:], AF.Identity, bias=cbias(-0.5 / 9.5),
                          scale=1.0 / 9.5)
            ew = wtile("ew")
            V.tensor_scalar(ew[:], wr[:], 1.0, 0.0, OP.min, OP.max)
            pw = wtile("pw")
            V.tensor_tensor(pw[:], beta[:], ew[:], OP.mult)
            V.tensor_tensor(pw[:], pw[:], is_obj[:], OP.mult)

            # --- energy loss pieces (pre-transcendental) ---
            ediff_r = wtile("ediff_r")
            V.tensor_tensor(ediff_r[:], sb["tE"][:], sb["pE"][:], OP.subtract)
            ediff = wtile("ediff")
            SC.activation(ediff[:], ediff_r[:], AF.Abs)
            ed2 = wtile("ed2")
            V.tensor_tensor(ed2[:], ediff[:], ediff[:], OP.mult)
            ed001 = wtile("ed001")
            SC.activation(ed001[:], ediff[:], AF.Copy, scale=0.001)

            # --- position loss pieces ---
            dpos = wtile("dpos", [P, T, 2])
            V.tensor_tensor(dpos[:], sb["tpos"][:], sb["ppos"][:], OP.subtract)
            V.tensor_tensor(dpos[:], dpos[:], dpos[:], OP.mult)
            d2p = wtile("d2p")
            V.tensor_tensor(d2p[:], dpos[:, :, 0], dpos[:, :, 1], OP.add)

            # --- timing loss pieces ---
            dtm = wtile("dtm")
            V.tensor_tensor(dtm[:], sb["ttime"][:], sb["ptime"][:], OP.subtract)
            adt = wtile("adt")
            SC.activation(adt[:], dtm[:], AF.Abs)
            dt2 = wtile("dt2")
            V.tensor_tensor(dt2[:], dtm[:], dtm[:], OP.mult)
            lint = wtile("lint")
            SC.activation(lint[:], adt[:], AF.Identity, bias=cbias(-4.0), scale=4.0)
            ltt = wtile("ltt", dtype=u8)
            V.tensor_scalar(ltt[:], adt[:], 2.0, None, OP.is_lt)
            ht = wtile("ht")
            V.select(ht[:], ltt[:], dt2[:], lint[:])
            yt = wtile("yt")
            SC.activation(yt[:], ht[:], AF.Copy, scale=1.0 / 6.0)

            # --- classification loss ---
            pid2 = wtile("pid2", [P, T, 6])
            V.tensor_tensor(pid2[:], sb["pid"][:], sb["pid"][:], OP.mult)
            cred = wtile("cred")
            V.tensor_reduce(cred[:], pid2[:], mybir.AxisListType.X, OP.add)

            # --- transcendental block: Exp, then Sqrt, then Ln (grouped to
            # limit ACT table swaps; per-engine issue is program-order) ---
            ex = wtile("ex")
            SC.activation(ex[:], ed2[:], AF.Exp, scale=-0.1)
            xp = wtile("xp")
            SC.activation(xp[:], d2p[:], AF.Sqrt, bias=cbias(0.01), scale=0.01)

            lnr = wtile("lnr")
            SC.activation(lnr[:], ratio[:], AF.Ln)
            # q = (0.5*ln(ratio))^2 + 0.1, zeroed on padding
            halfln = wtile("halfln")
            SC.activation(halfln[:], lnr[:], AF.Copy, scale=0.5)
            q = wtile("q")
            V.tensor_tensor(q[:], halfln[:], halfln[:], OP.mult)
            V.scalar_tensor_tensor(q[:], q[:], 0.1, sb["valid"][:],
                                   OP.add, OP.mult)

            # energy softclip
            ye = wtile("ye")
            V.tensor_tensor(ye[:], ex[:], ed001[:], OP.add)
            lnye = wtile("lnye")
            SC.activation(lnye[:], ye[:], AF.Ln, bias=cbias(1.0))
            gte = wtile("gte", dtype=u8)
            V.tensor_scalar(gte[:], ye[:], 1.0, None, OP.is_gt)
            esc = wtile("esc")
            V.select(esc[:], gte[:], lnye[:], ye[:])

            # position huber + softclip
            xp2 = wtile("xp2")
            V.tensor_tensor(xp2[:], xp[:], xp[:], OP.mult)
            linp = wtile("linp")
            SC.activation(linp[:], xp[:], AF.Identity, bias=cbias(-100.0), scale=20.0)
            ltp = wtile("ltp", dtype=u8)
            V.tensor_scalar(ltp[:], xp[:], 10.0, None, OP.is_lt)
            hp = wtile("hp")
            V.select(hp[:], ltp[:], xp2[:], linp[:])
            yp = wtile("yp")
            SC.activation(yp[:], hp[:], AF.Copy, scale=1.0 / 3.0)
            lnyp = wtile("lnyp")
            SC.activation(lnyp[:], yp[:], AF.Ln, bias=cbias(1.0))
            gtp = wtile("gtp", dtype=u8)
            V.tensor_scalar(gtp[:], yp[:], 1.0, None, OP.is_gt)
            psc = wtile("psc")
            V.select(psc[:], gtp[:], lnyp[:], yp[:])

            # timing softclip
            lnyt = wtile("lnyt")
            SC.activation(lnyt[:], yt[:], AF.Ln, bias=cbias(1.0))
            gtt = wtile("gtt", dtype=u8)
            V.tensor_scalar(gtt[:], yt[:], 1.0, None, OP.is_gt)
            tsc = wtile("tsc")
            V.select(tsc[:], gtt[:], lnyt[:], yt[:])

            # payload = 10*esc + 3*psc + 6*tsc + (1e-8/6)*cred
            esc10 = wtile("esc10")
            SC.activation(esc10[:], esc[:], AF.Copy, scale=10.0)
            pay = wtile("pay")
            V.scalar_tensor_tensor(pay[:], psc[:], 3.0, esc10[:],
                                   OP.mult, OP.add)
            V.scalar_tensor_tensor(pay[:], tsc[:], 6.0, pay[:],
                                   OP.mult, OP.add)
            V.scalar_tensor_tensor(pay[:], cred[:], 1e-8 / 6.0, pay[:],
                                   OP.mult, OP.add)
            paypw = wtile("paypw")
            V.tensor_tensor(paypw[:], pay[:], pw[:], OP.mult)

            # selection rhs: [x0, x1, q]
            sel3 = wtile("sel3", [P, T, 3])
            SC.activation(sel3[:, :, 0:2], sb["cc"][:], AF.Copy)
            V.tensor_copy(sel3[:, :, 2], q[:])

            # d2-matmul lhsT quantities [-2x0, -2x1, 1, |x|^2] packed [P,T,4]
            prep4 = wtile("prep4", [P, T, 4])
            SC.activation(prep4[:, :, 0:2], sb["cc"][:], AF.Copy, scale=-2.0)
            V.memset(prep4[:, :, 2], 1.0)
            ccsq = wtile("ccsq", [P, T, 2])
            V.tensor_tensor(ccsq[:], sb["cc"][:], sb["cc"][:], OP.mult)
            V.tensor_tensor(prep4[:, :, 3], ccsq[:, :, 0], ccsq[:, :, 1],
                            OP.add)

            # extras: [noise*beta, noise, |x|^2, q] free-reduced to [P,4]
            extras = io.tile([P, 4], f32, name="extras")
            nb_t = wtile("nb_t")
            V.tensor_tensor(nb_t[:], is_noise[:], beta[:], OP.mult)
            V.tensor_reduce(extras[:, 0:1], nb_t[:], mybir.AxisListType.X, OP.add)
            V.tensor_reduce(extras[:, 1:2], is_noise[:], mybir.AxisListType.X, OP.add)
            V.tensor_reduce(extras[:, 2:3], prep4[:, :, 3], mybir.AxisListType.X, OP.add)
            V.tensor_reduce(extras[:, 3:4], q[:], mybir.AxisListType.X, OP.add)

            # ---------- P2: local per-object beta max ----------
            runmax = io.tile([P, K], f32, name="runmax")
            V.memset(runmax[:], 0.0)
            with tc.tile_pool(name="bmp", bufs=3) as bmp:
                for t in range(T):
                    bm = bmp.tile([P, K], f32, name="bm")
                    V.tensor_scalar(bm[:], iotaF[:], sb["tidx"][:, t:t + 1],
                                    beta[:, t:t + 1], OP.is_equal, OP.mult)
                    V.tensor_tensor(runmax[:], runmax[:], bm[:], OP.max)

            # partition-reduce runmax -> Bloc [128,4] (k = 128*b + p)
            Bloc = io.tile([P, KB], f32, name="Bloc")
            for b in range(KB):
                tp = psA.tile([P, P], f32, name="tpose", tag="tpose")
                nc.tensor.transpose(tp[:], runmax[:, b * P:(b + 1) * P], ident[:])
                V.reduce_max(Bloc[:, b:b + 1], tp[:], axis=mybir.AxisListType.X)

            # free-layout broadcast copy of Bloc: BlocB[p, k] = Bloc[k%128, k//128]
            BlocF = io.tile([1, K], f32, name="BlocF")
            for b in range(KB):
                nc.sync.dma_start(BlocF[0:1, b * P:(b + 1) * P], Bloc[:, b:b + 1])
            # broadcast [1,K] across partitions via PE: ones[1,P].T @ BlocF
            BlocB = io.tile([P, K], f32, name="BlocB")
            with tc.tile_pool(name="bcp", bufs=1, space="PSUM") as bcp:
                blocps = bcp.tile([P, K], f32, name="blocps")
                nc.tensor.matmul(blocps[:], onesrow[:], BlocF[:],
                                 start=True, stop=True)
                SC.activation(BlocB[:], blocps[:], AF.Copy)

            # ---------- P3: selection segment-sums ----------
            with (
                tc.tile_pool(name="selpp", bufs=1, space="PSUM") as selpp,
                tc.tile_pool(name="bmp3", bufs=3) as bmp3,
            ):
                selP = selpp.tile([P, KB, 3], f32, name="selP")
                V.memset(selP[:], 0.0)
                for t in range(T):
                    bm = bmp3.tile([P, K], f32, name="bm3")
                    V.tensor_scalar(bm[:], iotaF[:], sb["tidx"][:, t:t + 1],
                                    beta[:, t:t + 1], OP.is_equal, OP.mult)
                    Isel = bmp3.tile([P, K], f32, name="Isel")
                    V.tensor_tensor(Isel[:], bm[:], BlocB[:], OP.is_equal)
                    for b in range(KB):
                        nc.tensor.matmul(selP[:, b, :],
                                         Isel[:, b * P:(b + 1) * P],
                                         sel3[:, t, :],
                                         start=False, stop=(t == T - 1),
                                         skip_group_check=True)

                selsb = io.tile([P, KB, 3], f32, name="selsb")
                SC.activation(selsb[:], selP[:], AF.Copy)

            # ---------- P4: AllReduce-max of Bloc, then AllReduce-add of
            # equality-gated selection sums (winner core contributes) ----------
            arm_in = dram.tile([1, K], f32, name="arm_in")
            arm_out = dram.tile([1, K], f32, name="arm_out", addr_space="Shared")
            nc.sync.dma_start(arm_in[0:1, :], Bloc[:, :])     # p-outer pack
            if cc_mode in ('all', 'first', 'two'):
                nc.gpsimd.collective_compute(
                    "AllReduce", OP.max,
                    replica_groups=[list(range(NCORES))],
                    ins=[arm_in[:]], outs=[arm_out[:]],
                )
            else:
                nc.sync.dma_start(arm_out[:], arm_in[:])
            BglobB = io.tile([P, KB], f32, name="BglobB")
            nc.sync.dma_start(
                BglobB[:],
                arm_out[0:1, :].rearrange("o (p b) -> (o p) b", p=P))
            keep = io.tile([P, KB], f32, name="keep")
            V.tensor_tensor(keep[:], Bloc[:], BglobB[:], OP.is_equal)
            sel_c = io.tile([P, KB, 3], f32, name="sel_c")
            for qq in range(3):
                V.tensor_tensor(sel_c[:, :, qq], selsb[:, :, qq], keep[:],
                                OP.mult)
            ar2_in = dram.tile([1, 3 * K], f32, name="ar2_in")
            ar2_out = dram.tile([1, 3 * K], f32, name="ar2_out",
                                addr_space="Shared")
            nc.sync.dma_start(ar2_in[0:1, :], sel_c[:])       # p-outer pack
            if cc_mode in ('all', 'two'):
                nc.gpsimd.collective_compute(
                    "AllReduce", OP.add,
                    replica_groups=[list(range(NCORES))],
                    ins=[ar2_in[:]], outs=[ar2_out[:]],
                )
            else:
                nc.sync.dma_start(ar2_out[:], ar2_in[:])
            sel_g = io.tile([P, KB, 3], f32, name="sel_g")
            nc.sync.dma_start(
                sel_g[:],
                ar2_out[0:1, :].rearrange("o (p r) -> (o p) r", p=P))

            BstarB = BglobB[:]          # [P,KB]  beta_alpha (max beta)
            xa0B = sel_g[:, :, 0]
            xa1B = sel_g[:, :, 1]
            qaB = sel_g[:, :, 2]
            xasqB = io.tile([P, KB], f32, name="xasqB")
            tmpa = io.tile([P, KB], f32, name="tmpa")
            V.tensor_tensor(tmpa[:], xa0B, xa0B, OP.mult)
            V.tensor_tensor(xasqB[:], xa1B, xa1B, OP.mult)
            V.tensor_tensor(xasqB[:], xasqB[:], tmpa[:], OP.add)

            # free-layout rhs for the d2 matmul: rows [xa0; xa1; |xa|^2; 1]
            rhsD2 = io.tile([4, K], f32, name="rhsD2")
            V.memset(rhsD2[:], 1.0)       # row 3 stays 1.0; rows 0-2 DMA'd over
            for b in range(KB):
                nc.sync.dma_start(rhsD2[0:1, b * P:(b + 1) * P],
                                  sel_g[:, b:b + 1, 0])
                nc.sync.dma_start(rhsD2[1:2, b * P:(b + 1) * P],
                                  sel_g[:, b:b + 1, 1])
                nc.sync.dma_start(rhsD2[2:3, b * P:(b + 1) * P],
                                  xasqB[:, b:b + 1])
            # fp32r-rounded copy for the fast-mode d2 matmul
            rhsD2r = io.tile([4, K], f32r, name="rhsD2r")
            SC.activation(rhsD2r[:], rhsD2[:], AF.Copy)
            # fp16 q for the repulsion matmul
            qh = io.tile([P, T], f16, name="qh")
            SC.activation(qh[:], q[:], AF.Copy)

            # transpose prep4 -> lhsT4 [4, T, 128]: quantity r on partition r,
            # tile t's stationary operand = lhsT4[0:4, t, :] (base partition 0)
            lhsT4 = io.tile([4, T, P], f32r, name="lhsT4")
            for r in range(4):
                tp = psA.tile([P, P], f32, name="tpose4", tag="tpose")
                nc.tensor.transpose(tp[0:T, :], prep4[:, :, r], ident[:])
                stage = io.tile([T, P], f32r, name=f"tstage{r}")
                SC.activation(stage[:], tp[0:T, :], AF.Copy)
                nc.sync.dma_start(lhsT4[r:r + 1, :, :], stage[:])

            # ---------- P5 loop 1: d2 block, rep row-sums, self-distance ----------
            gstD = io.tile([P, T], f32, name="gstD")       # (beta+1)*d2_self
            repP = accp.tile([1, K], f32, name="repP")
            V.memset(repP[:], 0.0)
            scr = io.tile([P, K], f32, name="scr")         # ttr full-out scratch
            KH = K // 2
            with (
                tc.tile_pool(name="d2pool", bufs=2, space="PSUM") as d2pool,
                tc.tile_pool(name="sp", bufs=3) as sp,
                tc.tile_pool(name="bmp5", bufs=3) as bmp5,
            ):
                for t in range(T):
                    lhs_t = lhsT4[0:4, t, :]
                    d2ps = d2pool.tile([P, K], f32, name="d2ps")
                    nc.tensor.matmul(d2ps[:], lhs_t, rhsD2r[:],
                                     start=True, stop=True)
                    bm = bmp5.tile([P, K], f32, name="bm5")
                    V.tensor_scalar(bm[:], iotaF[:], sb["tidx"][:, t:t + 1],
                                    betap1[:, t:t + 1], OP.is_equal, OP.mult)
                    sS = sp.tile([P, K], f16, name="sS")
                    SC.activation(sS[:], d2ps[:], AF.Sqrt, bias=cbias(SQ_BIAS))
                    smv = sp.tile([P, K], f16, name="smv")
                    V.tensor_scalar(smv[:], sS[:], 1.0, None, OP.min)
                    nc.tensor.matmul(repP[:], qh[:, t:t + 1], smv[:],
                                     start=False, stop=(t == T - 1),
                                     skip_group_check=True)
                    V.scalar_tensor_tensor(
                        scr[:], bm[:], 1.0, d2ps[:], OP.bypass, OP.mult,
                        accum_out=gstD[:, t:t + 1])

            # ---------- global per-hit math for segment rhs ----------
            qrb = wtile("qrb")
            V.tensor_tensor(qrb[:], q[:], rb1[:], OP.mult)
            G2 = wtile("G2")                    # d2_self
            V.tensor_tensor(G2[:], gstD[:], rb1[:], OP.mult)
            rhs_seg = io.tile([P, T, 6], f32, name="rhs_seg")
            # att' = q*d2_self/(b+1)
            V.tensor_tensor(rhs_seg[:, :, 0], G2[:], qrb[:], OP.mult)
            s2 = wtile("s2")
            SC.activation(s2[:], G2[:], AF.Sqrt, bias=cbias(SQ_BIAS))
            V.tensor_scalar(s2[:], s2[:], 1.0, None, OP.min)
            # qmin' = q*min(s,1)/(b+1)
            V.tensor_tensor(rhs_seg[:, :, 1], s2[:], qrb[:], OP.mult)
            V.tensor_tensor(rhs_seg[:, :, 2], sb["valid"][:], rb1[:], OP.mult)
            V.tensor_tensor(rhs_seg[:, :, 3], pw[:], rb1[:], OP.mult)
            V.tensor_tensor(rhs_seg[:, :, 4], paypw[:], rb1[:], OP.mult)
            V.tensor_copy(rhs_seg[:, :, 5], qrb[:])

            # ---------- P5 loop 2: segment sums with lhsT = bm ----------
            segP = accp.tile([P, KB, 6], f32, name="segP")
            V.memset(segP[:], 0.0)
            with tc.tile_pool(name="bmp6", bufs=3) as bmp6:
                for t in range(T):
                    bm = bmp6.tile([P, K], f32, name="bm6")
                    V.tensor_scalar(bm[:], iotaF[:], sb["tidx"][:, t:t + 1],
                                    betap1[:, t:t + 1], OP.is_equal, OP.mult)
                    for b in range(KB):
                        nc.tensor.matmul(segP[:, b, :],
                                         bm[:, b * P:(b + 1) * P],
                                         rhs_seg[:, t, :],
                                         start=False, stop=(t == T - 1),
                                         skip_group_check=True)

            # ---------- P6: AllReduce of partials ----------
            segsb = io.tile([P, KB, 6], f32, name="segsb")
            SC.activation(segsb[:], segP[:], AF.Copy)
            repsb = io.tile([1, K], f32, name="repsb")
            SC.activation(repsb[:], repP[:], AF.Copy)

            NSEG = P * KB * 6
            NTOT = NSEG + K + 4 * P
            ar_in = dram.tile([1, NTOT], f32, name="ar_in")
            ar_out = dram.tile([1, NTOT], f32, name="ar_out", addr_space="Shared")
            nc.sync.dma_start(ar_in[0:1, 0:NSEG], segsb[:])
            nc.sync.dma_start(ar_in[0:1, NSEG:NSEG + K], repsb[:])
            nc.sync.dma_start(ar_in[0:1, NSEG + K:NTOT], extras[:])
            if cc_mode == 'all':
                nc.gpsimd.collective_compute(
                    "AllReduce", OP.add,
                    replica_groups=[list(range(NCORES))],
                    ins=[ar_in[:]], outs=[ar_out[:]],
                )
            else:
                nc.sync.dma_start(ar_out[:], ar_in[:])
            seg_g = io.tile([P, KB, 6], f32, name="seg_g")
            nc.sync.dma_start(
                seg_g[:],
                ar_out[0:1, 0:NSEG].rearrange("o (p r) -> (o p) r", p=P))
            extras_g = io.tile([P, 4], f32, name="extras_g")
            nc.sync.dma_start(
                extras_g[:],
                ar_out[0:1, NSEG + K:NTOT].rearrange("o (p r) -> (o p) r", p=P))
            repB = io.tile([P, KB], f32, name="repB")
            for b in range(KB):
                nc.sync.dma_start(
                    repB[:, b:b + 1],
                    ar_out[0:1, NSEG + b * P:NSEG + (b + 1) * P])

            # ---------- P7: assembly ----------
            # scalars: [nb, nn, xsq, qsum]
            scpp = tc.tile_pool(name="scpp", bufs=1, space="PSUM")
            scp = scpp.__enter__()
            sc1P = scp.tile([1, 4], f32, name="sc1P")
            nc.tensor.matmul(sc1P[:], onescol[:], extras_g[:],
                             start=True, stop=True)
            sc1 = io.tile([1, 4], f32, name="sc1")
            SC.activation(sc1[:], sc1P[:], AF.Copy)
            qsum_col = io.tile([P, 1], f32, name="qsum_col")
            qsps = scp.tile([P, 1], f32, name="qsps")
            nc.tensor.matmul(qsps[:], onesrow[:], sc1[0:1, 3:4],
                             start=True, stop=True)
            SC.activation(qsum_col[:], qsps[:], AF.Copy)

            attseg = seg_g[:, :, 0]
            qminseg = seg_g[:, :, 1]
            count = seg_g[:, :, 2]
            pwseg = seg_g[:, :, 3]
            payseg = seg_g[:, :, 4]
            qseg = seg_g[:, :, 5]

            def ntile(name):
                return io.tile([P, KB], f32, name=name)

            has = ntile("has")
            V.tensor_scalar(has[:], count[:], 0.0, None, OP.is_gt)
            rc = ntile("rc")        # 1/(count+eps)
            V.tensor_scalar(rc[:], count[:], EPS, None, OP.add)
            V.reciprocal(rc[:], rc[:])
            rnc = ntile("rnc")      # 1/(N-count+eps)
            V.tensor_scalar(rnc[:], count[:], -1.0, float(N) + EPS,
                            OP.mult, OP.add)
            V.reciprocal(rnc[:], rnc[:])

            la = ntile("la")        # qa*attseg/(count+eps) * has
            V.tensor_tensor(la[:], attseg[:], qaB, OP.mult)
            V.tensor_tensor(la[:], la[:], rc[:], OP.mult)
            V.tensor_tensor(la[:], la[:], has[:], OP.mult)

            # repfull_k = qsum_tot - repmm_k ; corr_k = qseg - qminseg
            # (repfull - corr) = qsum - repB - qseg + qminseg
            lr = ntile("lr")
            V.tensor_scalar(lr[:], repB[:], qsum_col[:], None, OP.subtract)
            V.tensor_tensor(lr[:], lr[:], qseg[:], OP.add)
            V.tensor_tensor(lr[:], lr[:], qminseg[:], OP.subtract)
            # now lr = repB - qsum + qseg - qminseg = -(repfull - corr)
            V.tensor_tensor(lr[:], lr[:], qaB, OP.mult)
            V.tensor_tensor(lr[:], lr[:], rnc[:], OP.mult)
            V.tensor_tensor(lr[:], lr[:], has[:], OP.mult)
            SC.activation(lr[:], lr[:], AF.Copy, scale=-1.0)

            lb = ntile("lb")        # has*(1 - beta_alpha)
            V.tensor_scalar(lb[:], BstarB, -1.0, 1.0, OP.mult, OP.add)
            V.tensor_tensor(lb[:], lb[:], has[:], OP.mult)

            lp = ntile("lp")        # has*paynum/(payden+eps)
            V.tensor_scalar(lp[:], pwseg[:], EPS, None, OP.add)
            V.reciprocal(lp[:], lp[:])
            V.tensor_tensor(lp[:], lp[:], payseg[:], OP.mult)
            V.tensor_tensor(lp[:], lp[:], has[:], OP.mult)

            asm = io.tile([P, 5], f32, name="asm")
            V.tensor_reduce(asm[:, 0:1], la[:], mybir.AxisListType.X, OP.add)
            V.tensor_reduce(asm[:, 1:2], lr[:], mybir.AxisListType.X, OP.add)
            V.tensor_reduce(asm[:, 2:3], lb[:], mybir.AxisListType.X, OP.add)
            V.tensor_reduce(asm[:, 3:4], lp[:], mybir.AxisListType.X, OP.add)
            V.tensor_reduce(asm[:, 4:5], has[:], mybir.AxisListType.X, OP.add)
            sc2P = scp.tile([1, 5], f32, name="sc2P")
            nc.tensor.matmul(sc2P[:], onescol[:], asm[:], start=True, stop=True)
            fin = io.tile([1, 5], f32, name="fin")
            SC.activation(fin[:], sc2P[:], AF.Copy)

            # total = (la+lr+lb+lp)/n_obj + nb/(nn+eps) + 0.001*xsq/(2N)
            s4 = io.tile([1, 1], f32, name="s4")
            V.tensor_reduce(s4[:], fin[0:1, 0:4], mybir.AxisListType.X, OP.add)
            nobj = io.tile([1, 1], f32, name="nobj")
            V.tensor_scalar(nobj[:], fin[0:1, 4:5], EPS, None, OP.add)
            V.reciprocal(nobj[:], nobj[:])
            tot = io.tile([1, 1], f32, name="tot")
            V.tensor_tensor(tot[:], s4[:], nobj[:], OP.mult)
            nden = io.tile([1, 1], f32, name="nden")
            V.tensor_scalar(nden[:], sc1[0:1, 1:2], EPS, None, OP.add)
            V.reciprocal(nden[:], nden[:])
            V.tensor_tensor(nden[:], nden[:], sc1[0:1, 0:1], OP.mult)
            V.tensor_tensor(tot[:], tot[:], nden[:], OP.add)
            lcc = io.tile([1, 1], f32, name="lcc")
            SC.activation(lcc[:], sc1[0:1, 2:3], AF.Copy,
                          scale=0.001 / (2.0 * N))
            V.tensor_tensor(tot[:], tot[:], lcc[:], OP.add)
            nc.sync.dma_start(out_d.ap(), tot[:])
            scpp.__exit__(None, None, None)

    nc.compile()
    return nc


def _build2():
    """Restructured kernel: 3 fused passes, fp16 one-hot segment matmuls
    with quantities as the stationary operand (one small LDW per tile),
    global-max selection without keep-gating, fp32r d2 matmul.

    Pass A: local per-object max of beta           -> AllReduce(max)
    Pass B: winner-equality selection (x,q at alpha) + alpha-independent
            segment sums (count, pw, pay*pw, q)    -> AllReduce(add)
    Pass C: N_local x K block: d2 matmul, hinge, repulsion row-sums,
            self-distance extraction, attraction/qmin segment sums
                                                   -> AllReduce(add)
    Assembly: identical final math on every core.
    """
    import concourse.bass as bass
    import concourse.bacc as bacc
    import concourse.mybir as mybir
    import concourse.tile as tile
    from concourse import masks

    f32 = mybir.dt.float32
    f32r = mybir.dt.float32r
    f16 = mybir.dt.float16
    i32 = mybir.dt.int32
    u8 = mybir.dt.uint8
    AF = mybir.ActivationFunctionType
    OP = mybir.AluOpType

    nc = bacc.Bacc("TRN2", target_bir_lowering=False, debug=False,
                   num_devices=NCORES)

    di = {}

    def din(name, shape):
        di[name] = nc.dram_tensor(name, shape, f32, kind="ExternalInput")
        return di[name]

    din("beta_r", [P, T])
    din("cc", [P, T, 2])
    din("pE", [P, T])
    din("ppos", [P, T, 2])
    din("ptime", [P, T])
    din("pid", [P, T, 6])
    din("tE", [P, T])
    din("tpos", [P, T, 2])
    din("ttime", [P, T])
    din("tidx", [P, T])
    din("valid", [P, T])
    out_d = nc.dram_tensor("out", [1, 1], f32, kind="ExternalOutput")

    with tile.TileContext(nc) as tc:
        with (
            tc.tile_pool(name="const", bufs=1) as cpool,
            tc.tile_pool(name="io", bufs=1) as io,
            tc.tile_pool(name="dram", bufs=1, space="DRAM") as dram,
            tc.tile_pool(name="psT", bufs=2, space="PSUM") as psT,
        ):
            # ---------- constants ----------
            ident = cpool.tile([P, P], f32)
            masks.make_identity(nc, ident[:])
            iotaI = cpool.tile([P, K], i32)
            nc.gpsimd.iota(iotaI[:], pattern=[[1, K]], base=0,
                           channel_multiplier=0)
            iotaF = cpool.tile([P, K], f32)
            nc.vector.tensor_copy(iotaF[:], iotaI[:])
            iotaH = cpool.tile([P, K], f16)
            nc.vector.tensor_copy(iotaH[:], iotaF[:])
            onescol = cpool.tile([P, 1], f32)
            nc.vector.memset(onescol[:], 1.0)
            onesrow = cpool.tile([1, P], f32)
            nc.vector.memset(onesrow[:], 1.0)

            _cb = {}

            def cbias(val):
                if val not in _cb:
                    ct = cpool.tile([P, 1], f32, name=f"cb{len(_cb)}")
                    nc.vector.memset(ct[:], val)
                    _cb[val] = ct
                return _cb[val][:]

            # ---------- load inputs ----------
            sb = {}
            for name, h in di.items():
                t_sb = io.tile(list(h.shape), f32, name=f"sb_{name}")
                nc.sync.dma_start(t_sb[:], h.ap())
                sb[name] = t_sb

            V = nc.vector
            SC = nc.scalar

            def wtile(name, shape=None, dtype=None):
                return io.tile(shape or [P, T], dtype or f32, name=name)

            # ---------- P1: per-hit prep ----------
            beta = wtile("beta")
            V.tensor_scalar(beta[:], sb["beta_r"][:], 1e-6, 1.0 - 1e-6,
                            OP.max, OP.min)
            betap1 = wtile("betap1")
            SC.activation(betap1[:], beta[:], AF.Identity, bias=cbias(1.0))
            onem = wtile("onem")
            SC.activation(onem[:], beta[:], AF.Identity, bias=cbias(1.0),
                          scale=-1.0)
            recm = wtile("recm")
            V.reciprocal(recm[:], onem[:])
            ratio = wtile("ratio")
            V.tensor_tensor(ratio[:], betap1[:], recm[:], OP.mult)

            is_obj = wtile("is_obj")
            V.tensor_scalar(is_obj[:], sb["tidx"][:], 0.0, None, OP.is_ge)
            is_noise = wtile("is_noise")
            V.tensor_scalar(is_noise[:], sb["tidx"][:], -1.0, None,
                            OP.is_equal)

            wr = wtile("wr")
            SC.activation(wr[:], sb["tE"][:], AF.Identity,
                          bias=cbias(-0.5 / 9.5), scale=1.0 / 9.5)
            ew = wtile("ew")
            V.tensor_scalar(ew[:], wr[:], 1.0, 0.0, OP.min, OP.max)
            pw = wtile("pw")
            V.tensor_tensor(pw[:], beta[:], ew[:], OP.mult)
            V.tensor_tensor(pw[:], pw[:], is_obj[:], OP.mult)

            ediff_r = wtile("ediff_r")
            V.tensor_tensor(ediff_r[:], sb["tE"][:], sb["pE"][:], OP.subtract)
            ediff = wtile("ediff")
            SC.activation(ediff[:], ediff_r[:], AF.Abs)
            ed2 = wtile("ed2")
            V.tensor_tensor(ed2[:], ediff[:], ediff[:], OP.mult)
            ed001 = wtile("ed001")
            SC.activation(ed001[:], ediff[:], AF.Copy, scale=0.001)

            dpos = wtile("dpos", [P, T, 2])
            V.tensor_tensor(dpos[:], sb["tpos"][:], sb["ppos"][:], OP.subtract)
            V.tensor_tensor(dpos[:], dpos[:], dpos[:], OP.mult)
            d2p = wtile("d2p")
            V.tensor_tensor(d2p[:], dpos[:, :, 0], dpos[:, :, 1], OP.add)

            dtm = wtile("dtm")
            V.tensor_tensor(dtm[:], sb["ttime"][:], sb["ptime"][:],
                            OP.subtract)
            adt = wtile("adt")
            SC.activation(adt[:], dtm[:], AF.Abs)
            dt2 = wtile("dt2")
            V.tensor_tensor(dt2[:], dtm[:], dtm[:], OP.mult)
            lint = wtile("lint")
            SC.activation(lint[:], adt[:], AF.Identity, bias=cbias(-4.0),
                          scale=4.0)
            ltt = wtile("ltt", dtype=u8)
            V.tensor_scalar(ltt[:], adt[:], 2.0, None, OP.is_lt)
            ht = wtile("ht")
            V.select(ht[:], ltt[:], dt2[:], lint[:])
            yt = wtile("yt")
            SC.activation(yt[:], ht[:], AF.Copy, scale=1.0 / 6.0)

            pid2 = wtile("pid2", [P, T, 6])
            V.tensor_tensor(pid2[:], sb["pid"][:], sb["pid"][:], OP.mult)
            cred = wtile("cred")
            V.tensor_reduce(cred[:], pid2[:], mybir.AxisListType.X, OP.add)

            ex = wtile("ex")
            SC.activation(ex[:], ed2[:], AF.Exp, scale=-0.1)
            xp = wtile("xp")
            SC.activation(xp[:], d2p[:], AF.Sqrt, bias=cbias(0.01), scale=0.01)

            lnr = wtile("lnr")
            SC.activation(lnr[:], ratio[:], AF.Ln)
            halfln = wtile("halfln")
            SC.activation(halfln[:], lnr[:], AF.Copy, scale=0.5)
            q = wtile("q")
            V.tensor_tensor(q[:], halfln[:], halfln[:], OP.mult)
            V.scalar_tensor_tensor(q[:], q[:], 0.1, sb["valid"][:],
                                   OP.add, OP.mult)

            ye = wtile("ye")
            V.tensor_tensor(ye[:], ex[:], ed001[:], OP.add)
            lnye = wtile("lnye")
            SC.activation(lnye[:], ye[:], AF.Ln, bias=cbias(1.0))
            gte = wtile("gte", dtype=u8)
            V.tensor_scalar(gte[:], ye[:], 1.0, None, OP.is_gt)
            esc = wtile("esc")
            V.select(esc[:], gte[:], lnye[:], ye[:])

            xp2 = wtile("xp2")
            V.tensor_tensor(xp2[:], xp[:], xp[:], OP.mult)
            linp = wtile("linp")
            SC.activation(linp[:], xp[:], AF.Identity, bias=cbias(-100.0),
                          scale=20.0)
            ltp = wtile("ltp", dtype=u8)
            V.tensor_scalar(ltp[:], xp[:], 10.0, None, OP.is_lt)
            hp = wtile("hp")
            V.select(hp[:], ltp[:], xp2[:], linp[:])
            yp = wtile("yp")
            SC.activation(yp[:], hp[:], AF.Copy, scale=1.0 / 3.0)
            lnyp = wtile("lnyp")
            SC.activation(lnyp[:], yp[:], AF.Ln, bias=cbias(1.0))
            gtp = wtile("gtp", dtype=u8)
            V.tensor_scalar(gtp[:], yp[:], 1.0, None, OP.is_gt)
            psc = wtile("psc")
            V.select(psc[:], gtp[:], lnyp[:], yp[:])

            lnyt = wtile("lnyt")
            SC.activation(lnyt[:], yt[:], AF.Ln, bias=cbias(1.0))
            gtt = wtile("gtt", dtype=u8)
            V.tensor_scalar(gtt[:], yt[:], 1.0, None, OP.is_gt)
            tsc = wtile("tsc")
            V.select(tsc[:], gtt[:], lnyt[:], yt[:])

            esc10 = wtile("esc10")
            SC.activation(esc10[:], esc[:], AF.Copy, scale=10.0)
            pay = wtile("pay")
            V.scalar_tensor_tensor(pay[:], psc[:], 3.0, esc10[:],
                                   OP.mult, OP.add)
            V.scalar_tensor_tensor(pay[:], tsc[:], 6.0, pay[:],
                                   OP.mult, OP.add)
            V.scalar_tensor_tensor(pay[:], cred[:], 1e-8 / 6.0, pay[:],
                                   OP.mult, OP.add)
            paypw = wtile("paypw")
            V.tensor_tensor(paypw[:], pay[:], pw[:], OP.mult)

            # fp16 stationary packs for the one-hot matmuls
            sel3h = io.tile([P, T, 3], f16, name="sel3h")
            SC.activation(sel3h[:, :, 0:2], sb["cc"][:], AF.Copy)
            V.tensor_copy(sel3h[:, :, 2], q[:])
            seg4h = io.tile([P, T, 4], f16, name="seg4h")
            V.memset(seg4h[:, :, 0], 1.0)
            V.tensor_copy(seg4h[:, :, 1], pw[:])
            V.tensor_copy(seg4h[:, :, 2], paypw[:])
            V.tensor_copy(seg4h[:, :, 3], q[:])

            # d2-matmul lhsT quantities [-2x0, -2x1, 1, |x|^2] packed [P,T,4]
            prep4 = wtile("prep4", [P, T, 4])
            SC.activation(prep4[:, :, 0:2], sb["cc"][:], AF.Copy, scale=-2.0)
            V.memset(prep4[:, :, 2], 1.0)
            ccsq = wtile("ccsq", [P, T, 2])
            V.tensor_tensor(ccsq[:], sb["cc"][:], sb["cc"][:], OP.mult)
            V.tensor_tensor(prep4[:, :, 3], ccsq[:, :, 0], ccsq[:, :, 1],
                            OP.add)

            # extras: [noise*beta, noise, |x|^2, q] free-reduced to [P,4]
            extras = io.tile([P, 4], f32, name="extras")
            nb_t = wtile("nb_t")
            V.tensor_tensor(nb_t[:], is_noise[:], beta[:], OP.mult)
            V.tensor_reduce(extras[:, 0:1], nb_t[:], mybir.AxisListType.X,
                            OP.add)
            V.tensor_reduce(extras[:, 1:2], is_noise[:], mybir.AxisListType.X,
                            OP.add)
            V.tensor_reduce(extras[:, 2:3], prep4[:, :, 3],
                            mybir.AxisListType.X, OP.add)
            V.tensor_reduce(extras[:, 3:4], q[:], mybir.AxisListType.X,
                            OP.add)

            # transpose prep4 -> lhsT4 [4, T, 128] (fp32r-rounded)
            lhsT4 = io.tile([4, T, P], f32r, name="lhsT4")
            for r in range(4):
                tp = psT.tile([P, P], f32, name="tpose4", tag="tpose")
                nc.tensor.transpose(tp[0:T, :], prep4[:, :, r], ident[:])
                stage = io.tile([T, P], f32r, name=f"tstage{r}")
                SC.activation(stage[:], tp[0:T, :], AF.Copy)
                nc.sync.dma_start(lhsT4[r:r + 1, :, :], stage[:])

            # ---------- Pass A: local per-object beta max ----------
            runmax = io.tile([P, K], f32, name="runmax")
            V.memset(runmax[:], 0.0)
            with tc.tile_pool(name="bmpA", bufs=3) as bmpA:
                for t in range(T):
                    bm = bmpA.tile([P, K], f32, name="bmA")
                    V.tensor_scalar(bm[:], iotaF[:], sb["tidx"][:, t:t + 1],
                                    beta[:, t:t + 1], OP.is_equal, OP.mult)
                    V.tensor_tensor(runmax[:], runmax[:], bm[:], OP.max)

            # partition-reduce runmax -> Bloc [128,KB] (k = 128*b + p)
            Bloc = io.tile([P, KB], f32, name="Bloc")
            for b in range(KB):
                tp = psT.tile([P, P], f32, name="tpose", tag="tpose")
                nc.tensor.transpose(tp[:], runmax[:, b * P:(b + 1) * P],
                                    ident[:])
                V.reduce_max(Bloc[:, b:b + 1], tp[:], axis=mybir.AxisListType.X)

            # ---------- AR1: AllReduce-max of Bloc ----------
            arm_in = dram.tile([1, K], f32, name="arm_in")
            arm_out = dram.tile([1, K], f32, name="arm_out",
                                addr_space="Shared")
            nc.sync.dma_start(arm_in[0:1, :], Bloc[:, :])     # p-outer pack
            nc.gpsimd.collective_compute(
                "AllReduce", OP.max,
                replica_groups=[list(range(NCORES))],
                ins=[arm_in[:]], outs=[arm_out[:]],
            )
            BglobB = io.tile([P, KB], f32, name="BglobB")
            nc.sync.dma_start(
                BglobB[:],
                arm_out[0:1, :].rearrange("o (p b) -> (o p) b", p=P))
            BlocF = io.tile([1, K], f32, name="BlocF")        # k-order
            nc.sync.dma_start(
                BlocF[0:1, :],
                arm_out[0:1, :].rearrange("o (p b) -> o (b p)", p=P))
            BlocB = io.tile([P, K], f32, name="BlocB")
            with tc.tile_pool(name="bcp", bufs=1, space="PSUM") as bcp:
                blocps = bcp.tile([P, K], f32, name="blocps")
                nc.tensor.matmul(blocps[:], onesrow[:], BlocF[:],
                                 start=True, stop=True)
                SC.activation(BlocB[:], blocps[:], AF.Copy)

            # ---------- Pass B: selection + alpha-independent seg sums ----
            NSEL = 3 * K
            NSEG4 = 4 * K
            NEX = 4 * P
            NT2 = NSEL + NSEG4 + NEX
            with (
                tc.tile_pool(name="selps", bufs=1, space="PSUM") as selps,
                tc.tile_pool(name="bmpB", bufs=3) as bmpB,
            ):
                selPS = selps.tile([3, K], f32, name="selPS")
                segPS = selps.tile([4, K], f32, name="segPS")
                V.memset(selPS[:], 0.0)
                V.memset(segPS[:], 0.0)
                for t in range(T):
                    oh = bmpB.tile([P, K], f16, name="ohB")
                    V.tensor_scalar(oh[:], iotaH[:], sb["tidx"][:, t:t + 1],
                                    None, OP.is_equal)
                    isel = bmpB.tile([P, K], f16, name="iselB")
                    V.scalar_tensor_tensor(isel[:], BlocB[:],
                                           beta[:, t:t + 1], oh[:],
                                           OP.is_equal, OP.mult)
                    nc.tensor.matmul(selPS[:], sel3h[:, t, :], isel[:],
                                     start=False, stop=(t == T - 1),
                                     skip_group_check=True)
                    nc.tensor.matmul(segPS[:], seg4h[:, t, :], oh[:],
                                     start=False, stop=(t == T - 1),
                                     skip_group_check=True)
                sel_sb = io.tile([3, K], f32, name="sel_sb")
                SC.activation(sel_sb[:], selPS[:], AF.Copy)
                seg_sb = io.tile([4, K], f32, name="seg_sb")
                SC.activation(seg_sb[:], segPS[:], AF.Copy)

            # ---------- AR2: AllReduce-add ----------
            ar2_in = dram.tile([1, NT2], f32, name="ar2_in")
            ar2_out = dram.tile([1, NT2], f32, name="ar2_out",
                                addr_space="Shared")
            nc.sync.dma_start(ar2_in[0:1, 0:NSEL], sel_sb[:])
            nc.sync.dma_start(ar2_in[0:1, NSEL:NSEL + NSEG4], seg_sb[:])
            nc.sync.dma_start(ar2_in[0:1, NSEL + NSEG4:NT2], extras[:])
            nc.gpsimd.collective_compute(
                "AllReduce", OP.add,
                replica_groups=[list(range(NCORES))],
                ins=[ar2_in[:]], outs=[ar2_out[:]],
            )

            # unpack: [P,KB] views (k = 128*b + p) for assembly
            def unpackB(name, off):
                tl = io.tile([P, KB], f32, name=name)
                nc.sync.dma_start(
                    tl[:],
                    ar2_out[0:1, off:off + K].rearrange(
                        "o (b p) -> (o p) b", p=P))
                return tl

            xa0B = unpackB("xa0B", 0)
            xa1B = unpackB("xa1B", K)
            qaB_t = unpackB("qaB", 2 * K)
            countB = unpackB("countB", NSEL + 0 * K)
            denB = unpackB("denB", NSEL + 1 * K)
            numB = unpackB("numB", NSEL + 2 * K)
            qsegB = unpackB("qsegB", NSEL + 3 * K)
            extras_g = io.tile([P, 4], f32, name="extras_g")
            nc.sync.dma_start(
                extras_g[:],
                ar2_out[0:1, NSEL + NSEG4:NT2].rearrange(
                    "o (p r) -> (o p) r", p=P))

            # rhsD2 [4, K] k-order: rows [xa0; xa1; |xa|^2; 1]
            rhsD2 = io.tile([4, K], f32, name="rhsD2")
            V.memset(rhsD2[:], 1.0)
            nc.sync.dma_start(
                rhsD2[0:2, :],
                ar2_out[0:1, 0:2 * K].rearrange("o (r k) -> (o r) k", r=2))
            xasqB = io.tile([P, KB], f32, name="xasqB")
            tmpa = io.tile([P, KB], f32, name="tmpa")
            V.tensor_tensor(tmpa[:], xa0B[:], xa0B[:], OP.mult)
            V.tensor_tensor(xasqB[:], xa1B[:], xa1B[:], OP.mult)
            V.tensor_tensor(xasqB[:], xasqB[:], tmpa[:], OP.add)
            nc.sync.dma_start(
                rhsD2[2:3, :].rearrange("o (b p) -> o (p b)", p=P),
                xasqB[:])
            rhsD2r = io.tile([4, K], f32r, name="rhsD2r")
            SC.activation(rhsD2r[:], rhsD2[:], AF.Copy)

            # ---------- Pass C: d2 block, rep row-sums, att/qmin segs ------
            gssT = io.tile([P, T], f16, name="gssT")      # s_self per hit
            att2h = io.tile([P, T, 2], f16, name="att2h")
            scr = io.tile([P, K], f16, name="scr")
            with (
                tc.tile_pool(name="d2pool", bufs=2, space="PSUM") as d2pool,
                tc.tile_pool(name="accps", bufs=1, space="PSUM") as accps,
                tc.tile_pool(name="spC", bufs=3) as spC,
            ):
                repPS = accps.tile([1, K], f32, name="repPS")
                att2PS = accps.tile([2, K], f32, name="att2PS")
                V.memset(repPS[:], 0.0)
                V.memset(att2PS[:], 0.0)
                for t in range(T):
                    d2ps = d2pool.tile([P, K], f32, name="d2ps")
                    nc.tensor.matmul(d2ps[:], lhsT4[0:4, t, :], rhsD2r[:],
                                     start=True, stop=True)
                    sS = spC.tile([P, K], f16, name="sS")
                    SC.activation(sS[:], d2ps[:], AF.Sqrt, bias=cbias(SQ_BIAS))
                    smv = spC.tile([P, K], f16, name="smv")
                    V.tensor_scalar(smv[:], sS[:], 1.0, None, OP.min)
                    nc.tensor.matmul(repPS[:], sel3h[:, t, 2:3], smv[:],
                                     start=False, stop=(t == T - 1),
                                     skip_group_check=True)
                    oh = spC.tile([P, K], f16, name="ohC")
                    V.tensor_scalar(oh[:], iotaH[:], sb["tidx"][:, t:t + 1],
                                    None, OP.is_equal)
                    # s_self extraction (one nonzero term per row)
                    V.scalar_tensor_tensor(scr[:], oh[:], 1.0, sS[:],
                                           OP.bypass, OP.mult,
                                           accum_out=gssT[:, t:t + 1])
                    # att2h cols: r0 = q*(s^2 - bias) = q*d2_self,
                    #             r1 = q*min(s,1)
                    s2c = spC.tile([P, 1], f32, name="s2c")
                    V.scalar_tensor_tensor(s2c[:], gssT[:, t:t + 1], 1.0,
                                           gssT[:, t:t + 1],
                                           OP.bypass, OP.mult)
                    V.scalar_tensor_tensor(att2h[:, t, 0:1], s2c[:],
                                           -SQ_BIAS, q[:, t:t + 1],
                                           OP.add, OP.mult)
                    V.scalar_tensor_tensor(att2h[:, t, 1:2],
                                           gssT[:, t:t + 1], 1.0,
                                           q[:, t:t + 1], OP.min, OP.mult)
                    nc.tensor.matmul(att2PS[:], att2h[:, t, :], oh[:],
                                     start=False, stop=(t == T - 1),
                                     skip_group_check=True)
                att2sb = io.tile([2, K], f32, name="att2sb")
                SC.activation(att2sb[:], att2PS[:], AF.Copy)
                repsb = io.tile([1, K], f32, name="repsb")
                SC.activation(repsb[:], repPS[:], AF.Copy)

            # ---------- AR3: AllReduce-add ----------
            NT3 = 3 * K
            ar3_in = dram.tile([1, NT3], f32, name="ar3_in")
            ar3_out = dram.tile([1, NT3], f32, name="ar3_out",
                                addr_space="Shared")
            nc.sync.dma_start(ar3_in[0:1, 0:2 * K], att2sb[:])
            nc.sync.dma_start(ar3_in[0:1, 2 * K:NT3], repsb[:])
            nc.gpsimd.collective_compute(
                "AllReduce", OP.add,
                replica_groups=[list(range(NCORES))],
                ins=[ar3_in[:]], outs=[ar3_out[:]],
            )

            def unpack3(name, off):
                tl = io.tile([P, KB], f32, name=name)
                nc.sync.dma_start(
                    tl[:],
                    ar3_out[0:1, off:off + K].rearrange(
                        "o (b p) -> (o p) b", p=P))
                return tl

            attB = unpack3("attB", 0)
            qminB = unpack3("qminB", K)
            repB = unpack3("repB", 2 * K)

            # ---------- Assembly ----------
            scpp = tc.tile_pool(name="scpp", bufs=1, space="PSUM")
            scp = scpp.__enter__()
            sc1P = scp.tile([1, 4], f32, name="sc1P")
            nc.tensor.matmul(sc1P[:], onescol[:], extras_g[:],
                             start=True, stop=True)
            sc1 = io.tile([1, 4], f32, name="sc1")
            SC.activation(sc1[:], sc1P[:], AF.Copy)
            qsum_col = io.tile([P, 1], f32, name="qsum_col")
            qsps = scp.tile([P, 1], f32, name="qsps")
            nc.tensor.matmul(qsps[:], onesrow[:], sc1[0:1, 3:4],
                             start=True, stop=True)
            SC.activation(qsum_col[:], qsps[:], AF.Copy)

            def ntile(name):
                return io.tile([P, KB], f32, name=name)

            has = ntile("has")
            V.tensor_scalar(has[:], countB[:], 0.0, None, OP.is_gt)
            rc = ntile("rc")
            V.tensor_scalar(rc[:], countB[:], EPS, None, OP.add)
            V.reciprocal(rc[:], rc[:])
            rnc = ntile("rnc")
            V.tensor_scalar(rnc[:], countB[:], -1.0, float(N) + EPS,
                            OP.mult, OP.add)
            V.reciprocal(rnc[:], rnc[:])

            la = ntile("la")
            V.tensor_tensor(la[:], attB[:], qaB_t[:], OP.mult)
            V.tensor_tensor(la[:], la[:], rc[:], OP.mult)
            V.tensor_tensor(la[:], la[:], has[:], OP.mult)

            # lr = (qsum - repB - qsegB + qminB) * qa * rnc * has
            lr = ntile("lr")
            V.tensor_scalar(lr[:], repB[:], qsum_col[:], None, OP.subtract)
            # lr = repB - qsum ; want qsum - repB - qseg + qmin
            V.tensor_tensor(lr[:], lr[:], qsegB[:], OP.add)
            V.tensor_tensor(lr[:], lr[:], qminB[:], OP.subtract)
            # lr = repB - qsum + qseg - qmin = -(qsum - repB - qseg + qmin)
            V.tensor_tensor(lr[:], lr[:], qaB_t[:], OP.mult)
            V.tensor_tensor(lr[:], lr[:], rnc[:], OP.mult)
            V.tensor_tensor(lr[:], lr[:], has[:], OP.mult)
            SC.activation(lr[:], lr[:], AF.Copy, scale=-1.0)

            lb = ntile("lb")
            V.tensor_scalar(lb[:], BglobB[:], -1.0, 1.0, OP.mult, OP.add)
            V.tensor_tensor(lb[:], lb[:], has[:], OP.mult)

            lp = ntile("lp")
            V.tensor_scalar(lp[:], denB[:], EPS, None, OP.add)
            V.reciprocal(lp[:], lp[:])
            V.tensor_tensor(lp[:], lp[:], numB[:], OP.mult)
            V.tensor_tensor(lp[:], lp[:], has[:], OP.mult)

            asm = io.tile([P, 5], f32, name="asm")
            V.tensor_reduce(asm[:, 0:1], la[:], mybir.AxisListType.X, OP.add)
            V.tensor_reduce(asm[:, 1:2], lr[:], mybir.AxisListType.X, OP.add)
            V.tensor_reduce(asm[:, 2:3], lb[:], mybir.AxisListType.X, OP.add)
            V.tensor_reduce(asm[:, 3:4], lp[:], mybir.AxisListType.X, OP.add)
            V.tensor_reduce(asm[:, 4:5], has[:], mybir.AxisListType.X, OP.add)
            sc2P = scp.tile([1, 5], f32, name="sc2P")
            nc.tensor.matmul(sc2P[:], onescol[:], asm[:], start=True,
                             stop=True)
            fin = io.tile([1, 5], f32, name="fin")
            SC.activation(fin[:], sc2P[:], AF.Copy)

            s4 = io.tile([1, 1], f32, name="s4")
            V.tensor_reduce(s4[:], fin[0:1, 0:4], mybir.AxisListType.X,
                            OP.add)
            nobj = io.tile([1, 1], f32, name="nobj")
            V.tensor_scalar(nobj[:], fin[0:1, 4:5], EPS, None, OP.add)
            V.reciprocal(nobj[:], nobj[:])
            tot = io.tile([1, 1], f32, name="tot")
            V.tensor_tensor(tot[:], s4[:], nobj[:], OP.mult)
            nden = io.tile([1, 1], f32, name="nden")
            V.tensor_scalar(nden[:], sc1[0:1, 1:2], EPS, None, OP.add)
            V.reciprocal(nden[:], nden[:])
            V.tensor_tensor(nden[:], nden[:], sc1[0:1, 0:1], OP.mult)
            V.tensor_tensor(tot[:], tot[:], nden[:], OP.add)
            lcc = io.tile([1, 1], f32, name="lcc")
            SC.activation(lcc[:], sc1[0:1, 2:3], AF.Copy,
                          scale=0.001 / (2.0 * N))
            V.tensor_tensor(tot[:], tot[:], lcc[:], OP.add)
            nc.sync.dma_start(out_d.ap(), tot[:])
            scpp.__exit__(None, None, None)

    nc.compile()
    return nc


def _host_prep(inputs):
    """Slice, pad and re-layout the full inputs into 8 per-core input maps."""
    def lay(a2):                       # [SP, w] -> [128, T, w]
        w = a2.shape[1]
        r = a2.reshape(T, P, w).transpose(1, 0, 2)
        return np.ascontiguousarray(r.astype(np.float32))

    in_maps = []
    for c in range(NCORES):
        sl = slice(c * S, (c + 1) * S)

        def pad(a, fill=0.0):
            out = np.full((SP, a.shape[1]), fill, np.float32)
            out[:S] = a[sl]
            return out

        tidx = np.full((SP, 1), -2.0, np.float32)
        tidx[:S, 0] = inputs["t_idx"][sl, 0].astype(np.float32)
        valid = np.zeros((SP, 1), np.float32)
        valid[:S] = 1.0
        m = {
            "beta_r": lay(pad(inputs["pred_beta"]))[:, :, 0],
            "cc": lay(pad(inputs["pred_ccoords"])),
            "pE": lay(pad(inputs["pred_energy"]))[:, :, 0],
            "ppos": lay(pad(inputs["pred_pos"])),
            "ptime": lay(pad(inputs["pred_time"]))[:, :, 0],
            "pid": lay(pad(inputs["pred_id"])),
            "tE": lay(pad(inputs["t_energy"]))[:, :, 0],
            "tpos": lay(pad(inputs["t_pos"])),
            "ttime": lay(pad(inputs["t_time"]))[:, :, 0],
            "tidx": lay(tidx)[:, :, 0],
            "valid": lay(valid)[:, :, 0],
        }
        m = {k: np.ascontiguousarray(v) for k, v in m.items()}
        in_maps.append(m)
    return in_maps


def _run(inputs, trace=False, tmpdir=None):
    from concourse import bass_utils
    if "nc" not in _CACHE:
        _CACHE["nc"] = _build()
    nc = _CACHE["nc"]
    in_maps = _host_prep(inputs)
    res = bass_utils.run_bass_kernel_spmd(
        nc, in_maps, core_ids=list(range(NCORES)), trace=trace, tmpdir=tmpdir)
    return res


def kernel(**inputs):
    res = _run(inputs, trace=False)
    val = np.float32(res.results[0]["out"][0, 0])
    return np.array(val, dtype=np.float32)[()]


if __name__ == "__main__":
    d = np.load("/tmp/inputs.npz")
    inp = {k: d[k] for k in d.files}
    print("kernel:", kernel(**inp))

